# revision 1
# baseline (speedup 1.0000x reference)
"""Trainium2 Bass kernel for CrossAttentionFusion over ragged segments.

Contract: kernel(**inputs) takes the FULL unsharded inputs (as produced by
setup_inputs()) and returns the FULL (N, C) float32 output.

Math (per segment b, rows [start_b, start_b + min(len_b, LMAX))):
    Q = Qf @ Wq.T + bq ; K = Kf @ Wk.T + bk ; V = Kf @ Wv.T + bv
    out = softmax(Q K^T / sqrt(C), masked to valid keys) @ V, padded rows zero.

Device strategy (8 NeuronCores, data-parallel over segments, 64 segs/core):
  - Host pads/gathers segments to a dense [spc*512, 128] bf16 layout per core,
    zero-filling K-feature rows of padded key slots and passing a 0/1 key mask.
  - Per segment on device (all matmuls bf16, fp32 PSUM accumulate):
      qfT,kfT   : DMA-transpose loads [C, tok]
      QT = WqT.T @ qfT            -> [co, tok]   (+bq via per-partition add)
      KT = WkT.T @ kfT            -> [co, tok]   (+bk)
      V  = kfT_chunk.T @ WvT      -> [tok, co]   (natural layout; 4 chunks)
      scoresT[m,l] = KT_chunk.T @ QT   (4 m-blocks -> [128, 2048] fp32 PSUM)
      E = exp(scale * scoresT)         (one ACT instr; padded keys give E=1)
      out[l,co] (+ denom) = E_chunk.T @ [V | keymask]  (16 accum matmuls)
      out = out * (1/denom)            (padded keys excluded exactly: V rows
                                        are 0 and mask column is 0)
  - Host scatters valid rows back and adds bv (softmax rows sum to 1).
"""
import math
import numpy as np
import ml_dtypes

import concourse.bass as bass
import concourse.tile as tile
from concourse import mybir
from concourse.bass_utils import run_bass_kernel_spmd

N_CORES = 8
C = 128
LMAX = 512
P = 128
BF = mybir.dt.bfloat16
F32 = mybir.dt.float32

_PROGRAM_CACHE = {}
LAST_EXEC_NS = None
LAST_WALL_NS = None

_MAX_SYNC = 1


def _install_ntff_shim():
    """Optional: register the NTFF profile hook missing from this image so
    run_bass_kernel_spmd(trace=True) can report HW exec time."""
    import sys, types
    if "antenv.axon_hooks" in sys.modules:
        return
    try:
        if "/root/.axon_site" not in sys.path:
            sys.path.insert(0, "/root/.axon_site")
        from trn_agent_boot.trn_boot import _ntff_profile_via_ctypes
        hook = _ntff_profile_via_ctypes("/opt/axon/libaxon_pjrt.so")
        if hook is None:
            return
        m = types.ModuleType("antenv.axon_hooks")
        m.get_axon_ntff_profile_hook = lambda: hook
        sys.modules["antenv.axon_hooks"] = m
    except Exception:
        pass


def _split_excess_sync(nc):
    """walrus (CoreV3 setupSyncWait) rejects >4 sem waits/updates on one
    instruction; move the excess onto preceding/following NoOps."""
    n = 0
    for f in nc.m.functions:
        for bb in f.blocks:
            il = bb.instructions
            k = 0
            while k < len(il):
                inst = il[k]
                si = inst.sync_info
                if si is not None and si.on_wait is not None \
                        and len(si.on_wait) > _MAX_SYNC:
                    w = list(si.on_wait)
                    si.on_wait = w[-_MAX_SYNC:]
                    pos = k
                    for j in range(0, len(w) - _MAX_SYNC, _MAX_SYNC):
                        nop = mybir.InstNoOp(
                            name=f"SPLITW-{n}", ins=[], outs=[])
                        n += 1
                        nop.engine = inst.engine
                        nop.sync_info = mybir.SyncInfo(
                            on_wait=w[j:j + _MAX_SYNC], on_update=[])
                        il.insert(pos, nop)
                        pos += 1
                        k += 1
                if si is not None and si.on_update is not None \
                        and len(si.on_update) > _MAX_SYNC:
                    u = list(si.on_update)
                    si.on_update = u[:_MAX_SYNC]
                    pos = k + 1
                    for j in range(_MAX_SYNC, len(u), _MAX_SYNC):
                        nop = mybir.InstNoOp(
                            name=f"SPLITU-{n}", ins=[], outs=[])
                        n += 1
                        nop.engine = inst.engine
                        nop.sync_info = mybir.SyncInfo(
                            on_wait=[], on_update=u[j:j + _MAX_SYNC])
                        il.insert(pos, nop)
                        pos += 1
                k += 1
    return n


def _build_program(spc, with_bias=False):
    """Build the SPMD Bass program for `spc` segments per core."""
    nc = bass.Bass()
    ntok = spc * LMAX

    qf = nc.dram_tensor("qf", [ntok, C], BF, kind="ExternalInput")
    kf = nc.dram_tensor("kf", [ntok, C], BF, kind="ExternalInput")
    wqt = nc.dram_tensor("wqt", [C, C], BF, kind="ExternalInput")
    wkt = nc.dram_tensor("wkt", [C, C], BF, kind="ExternalInput")
    wvt = nc.dram_tensor("wvt", [C, C], BF, kind="ExternalInput")
    bq = nc.dram_tensor("bq", [C, 1], F32, kind="ExternalInput")
    bk = nc.dram_tensor("bk", [C, 1], F32, kind="ExternalInput")
    # keymask[p, s*4 + mb] = 1.0 if key token mb*128+p of segment s is valid
    km = nc.dram_tensor("km", [P, spc * 4], BF, kind="ExternalInput")
    out = nc.dram_tensor("out", [ntok, C], F32, kind="ExternalOutput")

    scale = 1.0 / math.sqrt(C)
    Exp = mybir.ActivationFunctionType.Exp

    with tile.TileContext(nc) as tc:
        with (
            tc.tile_pool(name="const", bufs=1) as constp,
            tc.tile_pool(name="feat", bufs=2) as featp,
            tc.tile_pool(name="proj", bufs=2) as projp,
            tc.tile_pool(name="ebuf", bufs=2) as ep,
            tc.tile_pool(name="obuf", bufs=3) as outp,
            tc.tile_pool(name="ps_qk", bufs=1, space="PSUM") as ps_qk,
            tc.tile_pool(name="ps_v", bufs=1, space="PSUM") as ps_v,
            tc.tile_pool(name="ps_sc", bufs=1, space="PSUM") as ps_sc,
            tc.tile_pool(name="ps_av", bufs=2, space="PSUM") as ps_av,
        ):
            wqt_sb = constp.tile([C, C], BF, tag="wqt")
            wkt_sb = constp.tile([C, C], BF, tag="wkt")
            wvt_sb = constp.tile([C, C], BF, tag="wvt")
            bq_sb = constp.tile([C, 1], F32, tag="bq")
            bk_sb = constp.tile([C, 1], F32, tag="bk")
            km_sb = constp.tile([P, spc * 4], BF, tag="km")
            nc.sync.dma_start(out=wqt_sb, in_=wqt[:, :])
            nc.sync.dma_start(out=wkt_sb, in_=wkt[:, :])
            nc.sync.dma_start(out=wvt_sb, in_=wvt[:, :])
            nc.sync.dma_start(out=bq_sb, in_=bq[:, :])
            nc.sync.dma_start(out=bk_sb, in_=bk[:, :])
            nc.sync.dma_start(out=km_sb, in_=km[:, :])

            SEGS_PER_BLK = 4
            n_blk = spc // SEGS_PER_BLK
            for blk in range(n_blk):
                t0 = blk * SEGS_PER_BLK * LMAX
                t1 = (blk + 1) * SEGS_PER_BLK * LMAX
                qfT = featp.tile([C, SEGS_PER_BLK * LMAX], BF, tag="qfT")
                nc.sync.dma_start_transpose(out=qfT, in_=qf[t0:t1, :])
                kfT = featp.tile([C, SEGS_PER_BLK * LMAX], BF, tag="kfT")
                nc.sync.dma_start_transpose(out=kfT, in_=kf[t0:t1, :])

                for j in range(SEGS_PER_BLK):
                    s = blk * SEGS_PER_BLK + j
                    qfT_s = qfT[:, j * LMAX:(j + 1) * LMAX]
                    kfT_s = kfT[:, j * LMAX:(j + 1) * LMAX]

                    # --- projections ---
                    qt_ps = ps_qk.tile([C, LMAX], F32, tag="qk")
                    nc.tensor.matmul(qt_ps, lhsT=wqt_sb, rhs=qfT_s,
                                     start=True, stop=True)
                    qt_sb = projp.tile([C, LMAX], BF, tag="qt")
                    if with_bias:
                        nc.vector.tensor_scalar_add(qt_sb, qt_ps, bq_sb)
                    else:
                        nc.vector.tensor_copy(qt_sb, qt_ps)

                    kt_ps = ps_qk.tile([C, LMAX], F32, tag="qk")
                    nc.tensor.matmul(kt_ps, lhsT=wkt_sb, rhs=kfT_s,
                                     start=True, stop=True)
                    kt_sb = projp.tile([C, LMAX], BF, tag="kt")
                    if with_bias:
                        nc.vector.tensor_scalar_add(kt_sb, kt_ps, bk_sb)
                    else:
                        nc.vector.tensor_copy(kt_sb, kt_ps)

                    v_ps = ps_v.tile([P, LMAX], F32, tag="v")
                    for tb in range(4):
                        nc.tensor.matmul(
                            v_ps[:, tb * C:(tb + 1) * C],
                            lhsT=kfT_s[:, tb * P:(tb + 1) * P],
                            rhs=wvt_sb, start=True, stop=True)
                    # V in [m, co] layout + key-mask column -> AV rhs
                    v_sb = projp.tile([P, 4, C + 1], BF, tag="v")
                    nc.scalar.copy(
                        out=v_sb[:, :, 0:C],
                        in_=v_ps.rearrange("p (t c) -> p t c", t=4))
                    nc.vector.tensor_copy(
                        out=v_sb[:, :, C:C + 1].rearrange(
                            "p m one -> p (m one)"),
                        in_=km_sb[:, s * 4:(s + 1) * 4])

                    # --- scores (transposed): [m, l] ---
                    sc_ps = ps_sc.tile([P, 4 * LMAX], F32, tag="sc")
                    for mb in range(4):
                        nc.tensor.matmul(
                            sc_ps[:, mb * LMAX:(mb + 1) * LMAX],
                            lhsT=kt_sb[:, mb * P:(mb + 1) * P],
                            rhs=qt_sb, start=True, stop=True)

                    e_sb = ep.tile([P, 4 * LMAX], BF, tag="e")
                    nc.scalar.activation(out=e_sb, in_=sc_ps, func=Exp,
                                         scale=scale)

                    # --- attention @ V (+ denominator in column C) ---
                    av_ps = [ps_av.tile([P, 2, C + 1], F32, tag="av",
                                        name=f"av{s}_{h}")
                             for h in range(2)]
                    for lb in range(4):
                        sl = av_ps[lb // 2][:, lb % 2, :]
                        for mb in range(4):
                            nc.tensor.matmul(
                                sl,
                                lhsT=e_sb[:, mb * LMAX + lb * P:
                                          mb * LMAX + (lb + 1) * P],
                                rhs=v_sb[:, mb, :],
                                start=(mb == 0), stop=(mb == 3))

                    # --- normalize + store ---
                    o_sb = outp.tile([P, 4, C], F32, tag="o")
                    r_sb = outp.tile([P, 4], F32, tag="r")
                    for lb in range(4):
                        nc.vector.reciprocal(
                            r_sb[:, lb:lb + 1],
                            av_ps[lb // 2][:, lb % 2, C:C + 1])
                    for lb in range(4):
                        nc.scalar.activation(
                            out=o_sb[:, lb, :],
                            in_=av_ps[lb // 2][:, lb % 2, 0:C],
                            func=mybir.ActivationFunctionType.Copy,
                            scale=r_sb[:, lb:lb + 1])
                    nc.sync.dma_start(
                        out=out[s * LMAX:(s + 1) * LMAX, :].rearrange(
                            "(lb p) c -> p lb c", p=P),
                        in_=o_sb)
    _split_excess_sync(nc)
    return nc


def kernel(Q_feature, K_feature, Wq, bq, Wk, bk, Wv, bv, offset):
    Q_feature = np.asarray(Q_feature, dtype=np.float32)
    K_feature = np.asarray(K_feature, dtype=np.float32)
    Wq = np.asarray(Wq, dtype=np.float32)
    Wk = np.asarray(Wk, dtype=np.float32)
    Wv = np.asarray(Wv, dtype=np.float32)
    bq = np.asarray(bq, dtype=np.float32)
    bk = np.asarray(bk, dtype=np.float32)
    bv = np.asarray(bv, dtype=np.float32)
    offset = np.asarray(offset, dtype=np.int64)

    N, Cdim = Q_feature.shape
    assert Cdim == C
    B = offset.shape[0]

    starts = np.concatenate([np.zeros(1, np.int64), offset[:-1]])
    lengths = offset - starts
    pos = np.arange(LMAX, dtype=np.int64)
    valid = pos[None, :] < lengths[:, None]          # (B, LMAX)

    # Pad segment count to a multiple of 8*4 (4 segments per DMA block).
    segs_per_core = -(-B // (N_CORES * 4)) * 4
    B_pad = segs_per_core * N_CORES

    idx = np.clip(starts[:, None] + pos[None, :], 0, N - 1)   # (B, LMAX)

    equal = (B * LMAX == N) and bool(
        np.array_equal(offset, np.arange(1, B + 1, dtype=np.int64) * LMAX))

    if equal and B == B_pad:
        qp = Q_feature.reshape(B, LMAX, C)
        kp = K_feature.reshape(B, LMAX, C)
        valid_all = True
    else:
        qp = Q_feature[idx]                                   # (B, LMAX, C)
        kp = np.where(valid[:, :, None], K_feature[idx], 0.0)
        valid_all = False
        if B != B_pad:
            pad = B_pad - B
            qp = np.concatenate([qp, np.zeros((pad, LMAX, C), np.float32)])
            kp = np.concatenate([kp, np.zeros((pad, LMAX, C), np.float32)])
            valid = np.concatenate([valid, np.zeros((pad, LMAX), bool)])

    qf = qp.reshape(B_pad * LMAX, C).astype(ml_dtypes.bfloat16)
    kf = kp.reshape(B_pad * LMAX, C).astype(ml_dtypes.bfloat16)

    # keymask[core][p, s*4+mb] = valid[seg, mb*128+p]
    vmask = np.ascontiguousarray(
        valid.reshape(B_pad, 4, P).transpose(2, 0, 1)        # (P, B_pad, 4)
        .reshape(P, B_pad * 4).astype(ml_dtypes.bfloat16))

    wqt = np.ascontiguousarray(Wq.T).astype(ml_dtypes.bfloat16)
    wkt = np.ascontiguousarray(Wk.T).astype(ml_dtypes.bfloat16)
    wvt = np.ascontiguousarray(Wv.T).astype(ml_dtypes.bfloat16)
    bq2 = np.ascontiguousarray(bq.reshape(C, 1))
    bk2 = np.ascontiguousarray(bk.reshape(C, 1))

    with_bias = bool(np.any(bq) or np.any(bk))
    key = (segs_per_core, with_bias)
    if key not in _PROGRAM_CACHE:
        _PROGRAM_CACHE[key] = _build_program(segs_per_core, with_bias)
    nc = _PROGRAM_CACHE[key]

    ntok = segs_per_core * LMAX
    in_maps = []
    for c in range(N_CORES):
        r0, r1 = c * ntok, (c + 1) * ntok
        s0, s1 = c * segs_per_core * 4, (c + 1) * segs_per_core * 4
        in_maps.append({
            "qf": np.ascontiguousarray(qf[r0:r1]),
            "kf": np.ascontiguousarray(kf[r0:r1]),
            "wqt": wqt, "wkt": wkt, "wvt": wvt,
            "bq": bq2, "bk": bk2,
            "km": np.ascontiguousarray(vmask[:, s0:s1]),
        })

    import os as _os
    import time as _time
    trace = bool(_os.environ.get("KERNEL_TRACE"))
    if trace:
        _install_ntff_shim()
    _t0 = _time.time()
    res = run_bass_kernel_spmd(nc, in_maps, list(range(N_CORES)),
                               trace=trace)
    global LAST_EXEC_NS, LAST_WALL_NS
    LAST_WALL_NS = int((_time.time() - _t0) * 1e9)
    LAST_EXEC_NS = res.exec_time_ns
    outp = np.concatenate([res.results[c]["out"] for c in range(N_CORES)])
    outp = outp.reshape(B_pad, LMAX, C)[:B]

    if valid_all:
        return np.ascontiguousarray(
            (outp + bv[None, None, :]).reshape(N, C).astype(np.float32))

    out_full = np.zeros((N, C), dtype=np.float32)
    v = valid[:B]
    out_full[idx[v]] = outp[v] + bv[None, :]
    return out_full



# revision 13
# speedup vs baseline: 1.7498x; 1.7498x over previous
"""Trainium2 Bass kernel for CrossAttentionFusion over ragged segments.

Contract: kernel(**inputs) takes the FULL unsharded inputs (as produced by
setup_inputs()) and returns the FULL (N, C) float32 output.

Math (per segment b, rows [start_b, start_b + min(len_b, LMAX))):
    Q = Qf @ Wq.T + bq ; K = Kf @ Wk.T + bk ; V = Kf @ Wv.T + bv
    out = softmax(Q K^T / sqrt(C), masked to valid keys) @ V, padded rows zero.

Device strategy (8 NeuronCores, data-parallel over segments, 64 segs/core):
  - Identity: Q K^T = qf (Wq^T Wk) kf^T, so host precomputes W2 = Wq^T Wk and
    the kernel never projects K. Non-zero bq/bk reduce (after the softmax
    row-cancellation) to a per-key additive score bias kb[m] = (Wk^T bq)·kf[m],
    computed on host and folded into the exp() bias operand.
  - Host pads/gathers segments to dense [C, spc*512] TRANSPOSED bf16 layouts
    (linear DMA; no on-device transpose), zero-filling invalid key rows, and
    passes a 0/1 fp8 key mask.
  - Per segment on device:
      Q'T = W2.T-form matmul -> [c2, l] (bf16 via DVE cast)
      V   = kfT_chunk.T @ WvT -> [m, co] (fp8 via DVE cast; kfT chunk is the
            stationary operand shared with the scores matmul)
      scoresT[m, l] = kfT_chunk.T @ Q'T  (2 halves of [128, 1024] fp32 PSUM)
      E   = exp(scale*scoresT - 2)       (ACT, fp8e4 out; -2 cancels in the
            softmax ratio and keeps exp() within fp8 range)
      out[l, co] (+ denom in col 128) = sum_m E^T [V | keymask]  via fp8
            DoubleRow matmuls (256-deep contraction, 2x PE rate)
      o_sb bf16 <- PSUM (Pool engine), DMA'd as [ntok, 129] numerator|denom
  - Host divides numerator by denominator, adds bv, scatters valid rows.
"""
import math
import numpy as np
import ml_dtypes

import concourse.bass as bass
import concourse.tile as tile
from concourse import mybir
from concourse.bass_utils import run_bass_kernel_spmd

N_CORES = 8
C = 128
LMAX = 512
P = 128
BF = mybir.dt.bfloat16
F32 = mybir.dt.float32
FP8 = mybir.dt.float8e4
NP_BF16 = ml_dtypes.bfloat16
NP_FP8 = ml_dtypes.float8_e4m3

EXP_SHIFT = -2.0  # uniform exp bias; cancels in softmax, keeps E in fp8 range

# AV matmul mode: "bf16" (16 plain matmuls), "fp8" (16 fp8 matmuls, 1x rate),
# "fp8dr" (8 fp8 DoubleRow matmuls, 2x rate)
import os as _os_mode
AV_MODE = _os_mode.environ.get("AV_MODE", "fp8dr")

_PROGRAM_CACHE = {}
LAST_EXEC_NS = None
LAST_WALL_NS = None

_MAX_SYNC = 1


def _install_ntff_shim():
    """Optional: register the NTFF profile hook missing from this image so
    run_bass_kernel_spmd(trace=True) can report HW exec time."""
    import sys, types
    if "antenv.axon_hooks" in sys.modules:
        return
    try:
        if "/root/.axon_site" not in sys.path:
            sys.path.insert(0, "/root/.axon_site")
        from trn_agent_boot.trn_boot import _ntff_profile_via_ctypes
        hook = _ntff_profile_via_ctypes("/opt/axon/libaxon_pjrt.so")
        if hook is None:
            return
        m = types.ModuleType("antenv.axon_hooks")
        m.get_axon_ntff_profile_hook = lambda: hook
        sys.modules["antenv.axon_hooks"] = m
    except Exception:
        pass


def _split_excess_sync(nc):
    """walrus (CoreV3 setupSyncWait) rejects >4 sem waits/updates on one
    instruction; move the excess onto preceding/following NoOps."""
    n = 0
    for f in nc.m.functions:
        for bb in f.blocks:
            il = bb.instructions
            k = 0
            while k < len(il):
                inst = il[k]
                si = inst.sync_info
                if si is not None and si.on_wait is not None \
                        and len(si.on_wait) > _MAX_SYNC:
                    w = list(si.on_wait)
                    si.on_wait = w[-_MAX_SYNC:]
                    pos = k
                    for j in range(0, len(w) - _MAX_SYNC, _MAX_SYNC):
                        nop = mybir.InstNoOp(
                            name=f"SPLITW-{n}", ins=[], outs=[])
                        n += 1
                        nop.engine = inst.engine
                        nop.sync_info = mybir.SyncInfo(
                            on_wait=w[j:j + _MAX_SYNC], on_update=[])
                        il.insert(pos, nop)
                        pos += 1
                        k += 1
                if si is not None and si.on_update is not None \
                        and len(si.on_update) > _MAX_SYNC:
                    u = list(si.on_update)
                    si.on_update = u[:_MAX_SYNC]
                    pos = k + 1
                    for j in range(_MAX_SYNC, len(u), _MAX_SYNC):
                        nop = mybir.InstNoOp(
                            name=f"SPLITU-{n}", ins=[], outs=[])
                        n += 1
                        nop.engine = inst.engine
                        nop.sync_info = mybir.SyncInfo(
                            on_wait=[], on_update=u[j:j + _MAX_SYNC])
                        il.insert(pos, nop)
                        pos += 1
                k += 1
    return n


def _build_program(spc, with_bias=False, av_mode=None):
    """Build the SPMD Bass program for `spc` segments per core."""
    if av_mode is None:
        av_mode = AV_MODE
    EDT = BF if av_mode == "bf16" else FP8
    nc = bass.Bass()
    ntok = spc * LMAX

    qft = nc.dram_tensor("qft", [C, ntok], BF, kind="ExternalInput")
    kft = nc.dram_tensor("kft", [C, ntok], BF, kind="ExternalInput")
    w2 = nc.dram_tensor("w2", [C, C], BF, kind="ExternalInput")
    wvt = nc.dram_tensor("wvt", [C, C], BF, kind="ExternalInput")
    # keymask[p, s*4 + mb] = 1.0 if key token mb*128+p of segment s is valid
    km = nc.dram_tensor("km", [P, spc * 4], EDT, kind="ExternalInput")
    # kb[p, s*4+mb] = scale*(Wk^T bq)·kf[token] + EXP_SHIFT  (exp bias operand)
    kb = nc.dram_tensor("kb", [P, spc * 4], F32, kind="ExternalInput")
    eb = nc.dram_tensor("eb", [P, 1], F32, kind="ExternalInput")
    out = nc.dram_tensor("out", [ntok, C + 1], BF, kind="ExternalOutput")

    scale = 1.0 / math.sqrt(C)
    Exp = mybir.ActivationFunctionType.Exp
    DR = mybir.MatmulPerfMode.DoubleRow

    with tile.TileContext(nc) as tc:
        with (
            tc.tile_pool(name="const", bufs=1) as constp,
            tc.tile_pool(name="feat",
                         bufs=int(_os_mode.environ.get("FEAT_BUFS", "2"))) as featp,
            tc.tile_pool(name="proj", bufs=2) as projp,
            tc.tile_pool(name="ebuf", bufs=2) as ep,
            tc.tile_pool(name="obuf", bufs=3) as outp,
            tc.tile_pool(name="ps_q", bufs=1, space="PSUM") as ps_q,
            tc.tile_pool(name="ps_v", bufs=1, space="PSUM") as ps_v,
            tc.tile_pool(name="ps_sc", bufs=2, space="PSUM") as ps_sc,
            tc.tile_pool(name="ps_av", bufs=1, space="PSUM") as ps_av,
        ):
            w2_sb = constp.tile([C, C], BF, tag="w2")
            wvt_sb = constp.tile([C, C], BF, tag="wvt")
            km_sb = constp.tile([P, spc * 4], EDT, tag="km")
            bias_sb = constp.tile([P, 1], F32, tag="bias")
            nc.sync.dma_start(out=bias_sb, in_=eb[:, :])
            nc.sync.dma_start(out=w2_sb, in_=w2[:, :])
            nc.sync.dma_start(out=wvt_sb, in_=wvt[:, :])
            nc.sync.dma_start(out=km_sb, in_=km[:, :])
            if with_bias:
                kb_sb = constp.tile([P, spc * 4], F32, tag="kb")
                nc.sync.dma_start(out=kb_sb, in_=kb[:, :])

            SEGS_PER_BLK = int(_os_mode.environ.get("SEGS_PER_BLK", "4"))
            n_blk = spc // SEGS_PER_BLK
            for blk in range(n_blk):
                t0 = blk * SEGS_PER_BLK * LMAX
                t1 = (blk + 1) * SEGS_PER_BLK * LMAX
                qfT = featp.tile([C, SEGS_PER_BLK * LMAX], BF, tag="qfT")
                nc.sync.dma_start(out=qfT, in_=qft[:, t0:t1])
                kfT = featp.tile([C, SEGS_PER_BLK * LMAX], BF, tag="kfT")
                nc.sync.dma_start(out=kfT, in_=kft[:, t0:t1])

                for j in range(SEGS_PER_BLK):
                    s = blk * SEGS_PER_BLK + j
                    qfT_s = qfT[:, j * LMAX:(j + 1) * LMAX]
                    kfT_s = kfT[:, j * LMAX:(j + 1) * LMAX]

                    # --- Q' = qf @ W2 projection, [c2, l] ---
                    q_ps = ps_q.tile([C, LMAX], F32, tag="q")
                    nc.tensor.matmul(q_ps, lhsT=w2_sb, rhs=qfT_s,
                                     start=True, stop=True)
                    qt_sb = projp.tile([C, LMAX], BF, tag="qt")
                    nc.vector.tensor_copy(qt_sb, q_ps)

                    # --- V projection, [m, co] (+ keymask col) in fp8 ---
                    v_ps = ps_v.tile([P, LMAX], F32, tag="v")
                    for tb in range(4):
                        nc.tensor.matmul(
                            v_ps[:, tb * C:(tb + 1) * C],
                            lhsT=kfT_s[:, tb * P:(tb + 1) * P],
                            rhs=wvt_sb, start=True, stop=True)
                    v8 = projp.tile([P, 4, C + 1], EDT, tag="v8")
                    nc.vector.tensor_copy(
                        out=v8[:, :, 0:C],
                        in_=v_ps.rearrange("p (t c) -> p t c", t=4))
                    nc.vector.tensor_copy(
                        out=v8[:, :, C:C + 1].rearrange(
                            "p m one -> p (m one)"),
                        in_=km_sb[:, s * 4:(s + 1) * 4])

                    # --- scoresT [m, l] + exp, two halves of 2 m-blocks ---
                    e8 = ep.tile([P, 4, LMAX], EDT, tag="e8")
                    for h in range(2):
                        sc = ps_sc.tile([P, 2, LMAX], F32, tag="sc")
                        for t in range(2):
                            mb = 2 * h + t
                            nc.tensor.matmul(
                                sc[:, t, :],
                                lhsT=kfT_s[:, mb * P:(mb + 1) * P],
                                rhs=qt_sb, start=True, stop=True)
                        if with_bias:
                            for t in range(2):
                                mb = 2 * h + t
                                nc.scalar.activation(
                                    out=e8[:, mb, :], in_=sc[:, t, :],
                                    func=Exp, scale=scale,
                                    bias=kb_sb[:, s * 4 + mb:s * 4 + mb + 1])
                        else:
                            nc.scalar.activation(
                                out=e8[:, 2 * h:2 * h + 2, :], in_=sc,
                                func=Exp, scale=scale, bias=bias_sb[:, 0:1])

                    # --- attention @ V (+ denominator in column C) ---
                    # fp8 DoubleRow: 256-deep contraction per matmul
                    av = [ps_av.tile([P, 2, C + 1], F32, tag="av",
                                     name=f"av{s}_{i}") for i in range(2)]
                    if av_mode == "fp8dr":
                        for lb in range(4):
                            sl = av[lb // 2][:, lb % 2, :]
                            for t, mbp in enumerate((0, 2)):
                                nc.tensor.matmul(
                                    sl,
                                    lhsT=e8[:, mbp:mbp + 2,
                                            lb * P:(lb + 1) * P],
                                    rhs=v8[:, mbp:mbp + 2, :],
                                    start=(t == 0), stop=(t == 1),
                                    perf_mode=DR)
                    else:
                        for lb in range(4):
                            sl = av[lb // 2][:, lb % 2, :]
                            for mb in range(4):
                                nc.tensor.matmul(
                                    sl,
                                    lhsT=e8[:, mb, lb * P:(lb + 1) * P],
                                    rhs=v8[:, mb, :],
                                    start=(mb == 0), stop=(mb == 3))

                    # --- drain numerator|denominator, store ---
                    # (GPSIMD cannot access PSUM; DVE drains both av tiles)
                    o_sb = outp.tile([P, 4, C + 1], BF, tag="o")
                    for i in range(2):
                        nc.vector.tensor_copy(
                            out=o_sb[:, 2 * i:2 * i + 2, :], in_=av[i])
                    nc.sync.dma_start(
                        out=out[s * LMAX:(s + 1) * LMAX, :].rearrange(
                            "(lb p) c -> p lb c", p=P),
                        in_=o_sb)
    _split_excess_sync(nc)
    return nc


def kernel(Q_feature, K_feature, Wq, bq, Wk, bk, Wv, bv, offset):
    Q_feature = np.asarray(Q_feature, dtype=np.float32)
    K_feature = np.asarray(K_feature, dtype=np.float32)
    Wq = np.asarray(Wq, dtype=np.float32)
    Wk = np.asarray(Wk, dtype=np.float32)
    Wv = np.asarray(Wv, dtype=np.float32)
    bq = np.asarray(bq, dtype=np.float32)
    bk = np.asarray(bk, dtype=np.float32)
    bv = np.asarray(bv, dtype=np.float32)
    offset = np.asarray(offset, dtype=np.int64)

    N, Cdim = Q_feature.shape
    assert Cdim == C
    B = offset.shape[0]

    starts = np.concatenate([np.zeros(1, np.int64), offset[:-1]])
    lengths = offset - starts
    pos = np.arange(LMAX, dtype=np.int64)
    valid = pos[None, :] < lengths[:, None]          # (B, LMAX)

    # Pad segment count to a multiple of 8*4 (4 segments per DMA block).
    segs_per_core = -(-B // (N_CORES * 4)) * 4
    B_pad = segs_per_core * N_CORES

    idx = np.clip(starts[:, None] + pos[None, :], 0, N - 1)   # (B, LMAX)

    equal = (B * LMAX == N) and bool(
        np.array_equal(offset, np.arange(1, B + 1, dtype=np.int64) * LMAX))

    if equal and B == B_pad:
        qp = Q_feature.reshape(B, LMAX, C)
        kp = K_feature.reshape(B, LMAX, C)
        valid_all = True
    else:
        qp = Q_feature[idx]                                   # (B, LMAX, C)
        kp = np.where(valid[:, :, None], K_feature[idx], 0.0)
        valid_all = False
        if B != B_pad:
            pad = B_pad - B
            qp = np.concatenate([qp, np.zeros((pad, LMAX, C), np.float32)])
            kp = np.concatenate([kp, np.zeros((pad, LMAX, C), np.float32)])
            valid = np.concatenate([valid, np.zeros((pad, LMAX), bool)])

    qf = qp.reshape(B_pad * LMAX, C)
    kf = kp.reshape(B_pad * LMAX, C)

    # keymask[p, s*4+mb] = valid[seg, mb*128+p]
    np_edt = NP_BF16 if AV_MODE == "bf16" else NP_FP8
    vmask = np.ascontiguousarray(
        valid.reshape(B_pad, 4, P).transpose(2, 0, 1)        # (P, B_pad, 4)
        .reshape(P, B_pad * 4).astype(np_edt))

    scale = 1.0 / math.sqrt(C)
    w2 = np.ascontiguousarray(Wq.T @ Wk).astype(NP_BF16)
    wvt = np.ascontiguousarray(Wv.T).astype(NP_BF16)

    with_bias = bool(np.any(bq) or np.any(bk))
    if with_bias:
        u = Wk.T @ bq                                         # (C,)
        kb_flat = scale * (kf @ u) + EXP_SHIFT                # (B_pad*LMAX,)
        kb_arr = np.ascontiguousarray(
            kb_flat.reshape(B_pad, 4, P).transpose(2, 0, 1)
            .reshape(P, B_pad * 4).astype(np.float32))
    else:
        kb_arr = np.zeros((P, B_pad * 4), np.float32)
    eb_arr = np.full((P, 1), EXP_SHIFT, np.float32)

    key = (segs_per_core, with_bias, AV_MODE)
    if key not in _PROGRAM_CACHE:
        _PROGRAM_CACHE[key] = _build_program(segs_per_core, with_bias)
    nc = _PROGRAM_CACHE[key]

    ntok = segs_per_core * LMAX
    in_maps = []
    for c in range(N_CORES):
        r0, r1 = c * ntok, (c + 1) * ntok
        s0, s1 = c * segs_per_core * 4, (c + 1) * segs_per_core * 4
        in_maps.append({
            "qft": np.ascontiguousarray(qf[r0:r1].T.astype(NP_BF16)),
            "kft": np.ascontiguousarray(kf[r0:r1].T.astype(NP_BF16)),
            "w2": w2, "wvt": wvt,
            "km": np.ascontiguousarray(vmask[:, s0:s1]),
            "kb": np.ascontiguousarray(kb_arr[:, s0:s1]),
            "eb": eb_arr,
        })

    import os as _os
    import time as _time
    trace = bool(_os.environ.get("KERNEL_TRACE"))
    if trace:
        _install_ntff_shim()
    _t0 = _time.time()
    res = run_bass_kernel_spmd(nc, in_maps, list(range(N_CORES)),
                               trace=trace)
    global LAST_EXEC_NS, LAST_WALL_NS
    LAST_WALL_NS = int((_time.time() - _t0) * 1e9)
    LAST_EXEC_NS = res.exec_time_ns
    raw = np.concatenate([res.results[c]["out"] for c in range(N_CORES)])
    raw = raw.astype(np.float32).reshape(B_pad, LMAX, C + 1)[:B]
    den = raw[:, :, C]
    den = np.where(den != 0.0, den, 1.0)
    outp = raw[:, :, :C] / den[:, :, None]

    if valid_all:
        return np.ascontiguousarray(
            (outp + bv[None, None, :]).reshape(N, C).astype(np.float32))

    out_full = np.zeros((N, C), dtype=np.float32)
    v = valid[:B]
    out_full[idx[v]] = outp[v] + bv[None, :]
    return out_full
